# revision 7
# baseline (speedup 1.0000x reference)
"""Trainium2 Bass kernel for nn_ChebySemi_70222715289681.

out = x + (f - conv3x3(x, kernelA)) / 6   (per-sample 3x3 kernels,
B=64 images of 512x512, fp32). Pure data parallel: batch sharded 8
samples per core across 8 NeuronCores, zero communication.

Per-core kernel (batch-transposed striped layout, bf16 wire format):
  Host ships x and f TRANSPOSED to [H, B, W] (and pre-casts to bf16;
  f pre-scaled by 1/6), so one image row across all 8 samples is 8KB
  contiguous in HBM. The image is processed in 5 row-stripes (4 x 126
  output rows + an 8-row tail); a stripe tile [128, 8*512] holds rows
  126s-1 .. 126s+126 one-row-per-partition -> every DMA moves ~1MB in
  8KB-per-partition descriptors (descriptor-rate / alignment bound
  otherwise: 1KB descriptors measured only ~77-147 GB/s/queue).
  With rows on adjacent partitions, the conv's three row taps collapse
  into one banded stationary matrix W_dj[p,c] = -k[p-c,dj]/6. Per
  (stripe, sample): 3 banded matmuls - the column taps dj handled by
  shifting the PSUM output window (dj=1 full width first with
  start=True, then dj=0 into cols 1.., dj=2 into cols ..511), which
  also kills the need for any column padding - plus one
  shifted-identity matmul adding x itself into PSUM. The DVE blend
  out = f/6 + psum is one fused scalar_tensor_tensor per
  (stripe, sample) (all operands partition-base 0; compute engines
  cannot access SBUF at partition base != 0 on this stack).
  Weights (24 banded + 1 shifted identity) are built host-side from
  kernelA and shipped as one [128, 25, 126] bf16 tensor. Loads issue
  on Sync (x) / Scalar (f) HWDGE rings, stores on the GpSimd SWDGE
  ring so a store waiting on compute never blocks a load. Output is
  stored bf16 [H, B, W]; host casts/untransposes to f32 [B,1,H,W].
"""
import numpy as np
import concourse.bass as bass
import concourse.mybir as mybir
from concourse.tile import TileContext
from concourse.bass_utils import run_bass_kernel_spmd

F32 = mybir.dt.float32
BF16 = mybir.dt.bfloat16
NPBF16 = mybir.dt.np(BF16)
ALU = mybir.AluOpType

N_CORES = 8
BPC = 8          # samples per core
H = W = 512
SH = 126         # output rows per full stripe
NS = 5           # stripes (4 full + tail)
TAIL = H - 4 * SH  # 8

_MAX_WAITS = 1


def _fixup_sync_waits(nc):
    """This walrus build rejects >1-2 sem-waits per instruction; move the
    excess onto NOPs inserted just before, on the same engine (same program
    order, so semantics are unchanged)."""
    n_fix = 0
    for fn in nc.m.functions:
        for blk in fn.blocks:
            out, changed = [], False
            for inst in blk.instructions:
                si = inst.sync_info
                waits = list(si.on_wait or []) if si is not None else []
                if len(waits) > _MAX_WAITS:
                    changed = True
                    n_fix += 1
                    for i in range(0, len(waits) - _MAX_WAITS, _MAX_WAITS):
                        nop = mybir.InstNoOp(
                            name=f"I-waitfix-{nc.next_id()}", ins=[], outs=[])
                        nop.engine = inst.engine
                        nop.sync_info = mybir.SyncInfo(
                            on_wait=waits[i:i + _MAX_WAITS], on_update=[])
                        out.append(nop)
                    inst.sync_info = mybir.SyncInfo(
                        on_wait=waits[len(waits) - _MAX_WAITS:],
                        on_update=list(si.on_update or []))
                out.append(inst)
            if changed:
                blk.instructions = out
    return n_fix


def gen_kernel(n_samples=BPC):
    nc = bass.Bass(target_bir_lowering=False)
    x = nc.dram_tensor("x", [H, n_samples, W], BF16, kind="ExternalInput")
    f = nc.dram_tensor("f", [H, n_samples, W], BF16, kind="ExternalInput")
    wts = nc.dram_tensor("wts", [128, 3 * n_samples + 1, SH], BF16,
                         kind="ExternalInput")
    out = nc.dram_tensor("out", [H, n_samples, W], BF16,
                         kind="ExternalOutput")

    BW = n_samples * W
    xid = 3 * n_samples  # shifted-identity slot (adds x into PSUM)

    with TileContext(nc) as tc:
        with tc.tile_pool(name="const", bufs=1) as cpool, \
             tc.tile_pool(name="data", bufs=3) as dpool, \
             tc.tile_pool(name="psum", bufs=8, space="PSUM") as ppool:

            wt = cpool.tile([128, 3 * n_samples + 1, SH], BF16)
            nc.sync.dma_start(out=wt[:], in_=wts[:, :, :])

            for s in range(NS):
                kdim = TAIL + 2 if s == 4 else 128
                cdim = TAIL if s == 4 else SH

                xs = dpool.tile([128, BW], BF16, tag="xs")
                fs = dpool.tile([128, BW], BF16, tag="fs")
                ol = dpool.tile([128, BW], BF16, tag="ol")

                # stripe tile partition p holds image row SH*s + p - 1
                # (row -1 / row H halos are memset-zero partitions).
                if s == 0:
                    nc.gpsimd.memset(xs[0:1, :], 0.0)
                    nc.sync.dma_start(
                        out=xs[1:128, :],
                        in_=x[0:127].rearrange("p b c -> p (b c)"))
                elif s == 4:
                    nc.gpsimd.memset(xs[0:TAIL + 2, :], 0.0)
                    nc.sync.dma_start(
                        out=xs[0:TAIL + 1, :],
                        in_=x[4 * SH - 1:H].rearrange("p b c -> p (b c)"))
                else:
                    nc.sync.dma_start(
                        out=xs[:],
                        in_=x[SH * s - 1:SH * s + 127].rearrange(
                            "p b c -> p (b c)"))
                nc.scalar.dma_start(
                    out=fs[0:cdim, :],
                    in_=f[SH * s:SH * s + cdim].rearrange("p b c -> p (b c)"))

                for b in range(n_samples):
                    ps = ppool.tile([128, W], F32, tag="ps")
                    o = b * W
                    # dj=1 (center column tap) first: full width, start=True
                    nc.tensor.matmul(
                        ps[0:cdim, :], wt[0:kdim, 3 * b + 1, 0:cdim],
                        xs[0:kdim, o:o + W], start=True, stop=False)
                    # dj=0: out col j taps x col j-1 -> psum window cols 1..
                    nc.tensor.matmul(
                        ps[0:cdim, 1:W], wt[0:kdim, 3 * b, 0:cdim],
                        xs[0:kdim, o:o + W - 1], start=False, stop=False)
                    # dj=2: out col j taps x col j+1 -> psum window cols ..511
                    nc.tensor.matmul(
                        ps[0:cdim, 0:W - 1], wt[0:kdim, 3 * b + 2, 0:cdim],
                        xs[0:kdim, o + 1:o + W], start=False, stop=False)
                    # + x itself via the shifted identity
                    nc.tensor.matmul(
                        ps[0:cdim, :], wt[0:kdim, xid, 0:cdim],
                        xs[0:kdim, o:o + W], start=False, stop=True)

                    # blend: out = f/6 + psum  (single fused DVE op)
                    nc.vector.scalar_tensor_tensor(
                        out=ol[0:cdim, o:o + W], in0=fs[0:cdim, o:o + W],
                        scalar=1.0, in1=ps[0:cdim, :],
                        op0=ALU.mult, op1=ALU.add)

                nc.gpsimd.dma_start(
                    out=out[SH * s:SH * s + cdim].rearrange(
                        "p b c -> p (b c)"),
                    in_=ol[0:cdim, :])
    return nc


def _make_wts(kA):
    """[128, 25, 126] bf16: slot 3b+dj holds the banded conv weight
    W[p, c] = -kA[b, 0, p-c, dj]/6 (p-c in 0..2); slot 24 the shifted
    identity delta(p == c+1) that adds x itself into PSUM."""
    w = np.zeros((128, 3 * BPC + 1, SH), np.float32)
    c = np.arange(SH)
    for b in range(BPC):
        for dj in range(3):
            for di in range(3):
                w[c + di, 3 * b + dj, c] = -kA[b, 0, di, dj] / 6.0
    w[c + 1, 3 * BPC, c] = 1.0
    return w.astype(NPBF16)


def _make_in_maps(x, f, kernelA):
    in_maps = []
    for cid in range(N_CORES):
        s = slice(cid * BPC, (cid + 1) * BPC)
        # [B, 1, H, W] -> [H, B, W]: one row across all samples is 8KB
        xt = np.ascontiguousarray(
            x[s, 0].transpose(1, 0, 2)).astype(NPBF16)
        ft = np.ascontiguousarray(
            (f[s, 0] * (1.0 / 6.0)).transpose(1, 0, 2)).astype(NPBF16)
        in_maps.append({"x": xt, "f": ft, "wts": _make_wts(kernelA[s])})
    return in_maps


def run_sharded(x, f, kernelA, trace=False):
    """Compile+run on 8 cores; returns (full output, BassKernelResults)."""
    x = np.asarray(x, dtype=np.float32)
    f = np.asarray(f, dtype=np.float32)
    kernelA = np.asarray(kernelA, dtype=np.float32)
    nc = gen_kernel()
    _fixup_sync_waits(nc)
    res = run_bass_kernel_spmd(nc, _make_in_maps(x, f, kernelA),
                               core_ids=list(range(N_CORES)), trace=trace)
    out = np.concatenate(
        [res.results[c]["out"].astype(np.float32)
         .transpose(1, 0, 2).reshape(BPC, 1, H, W)
         for c in range(N_CORES)], axis=0)
    return out, res


def kernel(x, f, kernelA):
    out, _ = run_sharded(x, f, kernelA, trace=False)
    return out


# revision 10
# speedup vs baseline: 1.6464x; 1.6464x over previous
"""Trainium2 Bass kernel for nn_ChebySemi_70222715289681.

out = x + (f - conv3x3(x, kernelA)) / 6   (per-sample 3x3 kernels,
B=64 images of 512x512, fp32). Pure data parallel: batch sharded 8
samples per core across 8 NeuronCores, zero communication.

Per-core kernel (batch-transposed striped layout, bf16 wire format):
  Host ships x and f TRANSPOSED to [H, B, W] (and pre-casts to bf16;
  f pre-scaled by 1/6), so one image row across all 8 samples is 8KB
  contiguous in HBM. The image is processed in 5 row-stripes (4 x 126
  output rows + an 8-row tail); a stripe tile [128, 8*512] holds rows
  126s-1 .. 126s+126 one-row-per-partition -> every DMA moves ~1MB in
  8KB-per-partition descriptors (descriptor-rate / alignment bound
  otherwise: 1KB descriptors measured only ~77-147 GB/s/queue).
  With rows on adjacent partitions, the conv's three row taps collapse
  into one banded stationary matrix W_dj[p,c] = -k[p-c,dj]/6. Per
  (stripe, sample): 3 banded matmuls - the column taps dj handled by
  shifting the PSUM output window (dj=1 full width first with
  start=True, then dj=0 into cols 1.., dj=2 into cols ..511), which
  also kills the need for any column padding - plus one
  shifted-identity matmul adding x itself into PSUM. The DVE blend
  out = f/6 + psum is one fused scalar_tensor_tensor per
  (stripe, sample) (all operands partition-base 0; compute engines
  cannot access SBUF at partition base != 0 on this stack).
  Weights (24 banded + 1 shifted identity) are built host-side from
  kernelA and shipped as one [128, 25, 126] bf16 tensor. Loads issue
  on Sync (x) / Scalar (f) HWDGE rings, stores on the GpSimd SWDGE
  ring so a store waiting on compute never blocks a load. Output is
  stored bf16 [H, B, W]; host casts/untransposes to f32 [B,1,H,W].
"""
import numpy as np
import concourse.bass as bass
import concourse.mybir as mybir
from concourse.tile import TileContext
from concourse.bass_utils import run_bass_kernel_spmd

F32 = mybir.dt.float32
BF16 = mybir.dt.bfloat16
NPBF16 = mybir.dt.np(BF16)
ALU = mybir.AluOpType

N_CORES = 8
BPC = 8          # samples per core
H = W = 512
SH = 126         # output rows per full stripe
NS = 5           # stripes (4 full + tail)
TAIL = H - 4 * SH  # 8

_MAX_WAITS = 1


def _fixup_sync_waits(nc):
    """This walrus build rejects >1-2 sem-waits per instruction; move the
    excess onto NOPs inserted just before, on the same engine (same program
    order, so semantics are unchanged)."""
    n_fix = 0
    for fn in nc.m.functions:
        for blk in fn.blocks:
            out, changed = [], False
            for inst in blk.instructions:
                si = inst.sync_info
                waits = list(si.on_wait or []) if si is not None else []
                if len(waits) > _MAX_WAITS:
                    changed = True
                    n_fix += 1
                    for i in range(0, len(waits) - _MAX_WAITS, _MAX_WAITS):
                        nop = mybir.InstNoOp(
                            name=f"I-waitfix-{nc.next_id()}", ins=[], outs=[])
                        nop.engine = inst.engine
                        nop.sync_info = mybir.SyncInfo(
                            on_wait=waits[i:i + _MAX_WAITS], on_update=[])
                        out.append(nop)
                    inst.sync_info = mybir.SyncInfo(
                        on_wait=waits[len(waits) - _MAX_WAITS:],
                        on_update=list(si.on_update or []))
                out.append(inst)
            if changed:
                blk.instructions = out
    return n_fix


def gen_kernel(n_samples=BPC):
    nc = bass.Bass(target_bir_lowering=False)
    # x is host-padded with a zero row on top and bottom ([H+2, B, W]) so
    # every stripe load covers a partition range starting at 0: a dst
    # partition range starting elsewhere (e.g. [1:128]) defeats the
    # DGE's per-engine descriptor split - all descriptors land on ONE
    # SDMA engine and the transfer serializes at ~27 GB/s.
    x = nc.dram_tensor("x", [H + 2, n_samples, W], BF16,
                       kind="ExternalInput")
    f = nc.dram_tensor("f", [H, n_samples, W], BF16, kind="ExternalInput")
    wts = nc.dram_tensor("wts", [128, 3 * n_samples + 1, SH], BF16,
                         kind="ExternalInput")
    out = nc.dram_tensor("out", [H, n_samples, W], BF16,
                         kind="ExternalOutput")

    BW = n_samples * W
    xid = 3 * n_samples  # shifted-identity slot (adds x into PSUM)

    with TileContext(nc) as tc:
        with tc.tile_pool(name="const", bufs=1) as cpool, \
             tc.tile_pool(name="data", bufs=3) as dpool, \
             tc.tile_pool(name="psum", bufs=8, space="PSUM") as ppool:

            wt = cpool.tile([128, 3 * n_samples + 1, SH], BF16)
            nc.sync.dma_start(out=wt[:], in_=wts[:, :, :])

            for s in range(NS):
                kdim = TAIL + 2 if s == 4 else 128
                cdim = TAIL if s == 4 else SH

                xs = dpool.tile([128, BW], BF16, tag="xs")
                fs = dpool.tile([128, BW], BF16, tag="fs")
                ol = dpool.tile([128, BW], BF16, tag="ol")

                # stripe tile partition p holds image row SH*s + p - 1
                # (= padded-x row SH*s + p; rows -1 and H are host zeros).
                nc.sync.dma_start(
                    out=xs[0:kdim, :],
                    in_=x[SH * s:SH * s + kdim].rearrange(
                        "p b c -> p (b c)"))
                nc.scalar.dma_start(
                    out=fs[0:cdim, :],
                    in_=f[SH * s:SH * s + cdim].rearrange("p b c -> p (b c)"))

                for b in range(n_samples):
                    ps = ppool.tile([128, W], F32, tag="ps")
                    o = b * W
                    # dj=1 (center column tap) first: full width, start=True
                    nc.tensor.matmul(
                        ps[0:cdim, :], wt[0:kdim, 3 * b + 1, 0:cdim],
                        xs[0:kdim, o:o + W], start=True, stop=False)
                    # dj=0: out col j taps x col j-1 -> psum window cols 1..
                    nc.tensor.matmul(
                        ps[0:cdim, 1:W], wt[0:kdim, 3 * b, 0:cdim],
                        xs[0:kdim, o:o + W - 1], start=False, stop=False)
                    # dj=2: out col j taps x col j+1 -> psum window cols ..511
                    nc.tensor.matmul(
                        ps[0:cdim, 0:W - 1], wt[0:kdim, 3 * b + 2, 0:cdim],
                        xs[0:kdim, o + 1:o + W], start=False, stop=False)
                    # + x itself via the shifted identity
                    nc.tensor.matmul(
                        ps[0:cdim, :], wt[0:kdim, xid, 0:cdim],
                        xs[0:kdim, o:o + W], start=False, stop=True)

                    # blend: out = f/6 + psum  (single fused DVE op)
                    nc.vector.scalar_tensor_tensor(
                        out=ol[0:cdim, o:o + W], in0=fs[0:cdim, o:o + W],
                        scalar=1.0, in1=ps[0:cdim, :],
                        op0=ALU.mult, op1=ALU.add)

                nc.gpsimd.dma_start(
                    out=out[SH * s:SH * s + cdim].rearrange(
                        "p b c -> p (b c)"),
                    in_=ol[0:cdim, :])
    return nc


def _make_wts(kA):
    """[128, 25, 126] bf16: slot 3b+dj holds the banded conv weight
    W[p, c] = -kA[b, 0, p-c, dj]/6 (p-c in 0..2); slot 24 the shifted
    identity delta(p == c+1) that adds x itself into PSUM."""
    w = np.zeros((128, 3 * BPC + 1, SH), np.float32)
    c = np.arange(SH)
    for b in range(BPC):
        for dj in range(3):
            for di in range(3):
                w[c + di, 3 * b + dj, c] = -kA[b, 0, di, dj] / 6.0
    w[c + 1, 3 * BPC, c] = 1.0
    return w.astype(NPBF16)


def _make_in_maps(x, f, kernelA):
    in_maps = []
    for cid in range(N_CORES):
        s = slice(cid * BPC, (cid + 1) * BPC)
        # [B, 1, H, W] -> [H+2, B, W]: one row across all samples is 8KB;
        # zero halo rows at top/bottom keep all loads partition-0-based.
        xt = np.zeros((H + 2, BPC, W), dtype=NPBF16)
        xt[1:H + 1] = x[s, 0].transpose(1, 0, 2).astype(NPBF16)
        ft = np.ascontiguousarray(
            (f[s, 0] * (1.0 / 6.0)).transpose(1, 0, 2)).astype(NPBF16)
        in_maps.append({"x": xt, "f": ft, "wts": _make_wts(kernelA[s])})
    return in_maps


def run_sharded(x, f, kernelA, trace=False):
    """Compile+run on 8 cores; returns (full output, BassKernelResults)."""
    x = np.asarray(x, dtype=np.float32)
    f = np.asarray(f, dtype=np.float32)
    kernelA = np.asarray(kernelA, dtype=np.float32)
    nc = gen_kernel()
    _fixup_sync_waits(nc)
    res = run_bass_kernel_spmd(nc, _make_in_maps(x, f, kernelA),
                               core_ids=list(range(N_CORES)), trace=trace)
    out = np.concatenate(
        [res.results[c]["out"].astype(np.float32)
         .transpose(1, 0, 2).reshape(BPC, 1, H, W)
         for c in range(N_CORES)], axis=0)
    return out, res


def kernel(x, f, kernelA):
    out, _ = run_sharded(x, f, kernelA, trace=False)
    return out
